# revision 12
# baseline (speedup 1.0000x reference)
"""Trainium2 Bass kernel for nn_BaltNet (2-layer ConvLSTM + decoder + MLP head).

Sharding: data-parallel over batch B=8 (one sample per NeuronCore) for the
recurrent conv part; FC1's [131072, 256] contraction is K-sharded 8 ways
(AllToAll of the decoder features, per-core partial matmul, ReduceScatter).

Layout notes
------------
Conv is computed as matmuls over a zero-padded spatial layout [C, 66, 68]
(1-row halo top/bottom, cols 2..65 interior) so every 3x3 tap is a pure
free-dim offset.  The three vertical taps (ky) are packed into the matmul
contraction dim by keeping row-shifted copies of the input stacked on
partitions; the three horizontal taps (kx) are separate accumulating matmul
passes with shifted column windows.

  A  [105, 66, 68]: layer-0 rhs, 3 groups of (h0[32] + x[3]) at ky=0,-1,+1
      (base group first: engine writes need 32-aligned partition starts)
  Ba [128, 66, 68]: layer-1 rhs, groups (h0+h1)[64] at ky=-1 (p0-63), ky=0
  Bb [ 64, 66, 68]: layer-1 rhs, group  (h0+h1)[64] at ky=+1

Gates: z = [i f o g] on 128 partitions; g-gate weights/bias pre-scaled x2 so
tanh(g) = 2*sigmoid(2g) - 1 and one Sigmoid covers all 128 partitions.
Everything 16-bit is fp16 (verified ~1.2e-3 end-to-end vs fp32 reference).
"""

import os
import sys

for _p in ("/opt/trn_rl_repo",):
    if _p not in sys.path and os.path.isdir(_p):
        sys.path.insert(0, _p)

import numpy as np

import concourse.bass as bass
import concourse.mybir as mybir
import concourse.tile as tile
from concourse import bacc
from concourse.bass_utils import run_bass_kernel_spmd

F16 = mybir.dt.float16
F32 = mybir.dt.float32
AF = mybir.ActivationFunctionType
OP = mybir.AluOpType

B, T, C, HID, H, W = 8, 24, 3, 32, 64, 64
G4 = 4 * HID            # 128 gate channels
PH, PW = H + 2, W + 4   # padded spatial: rows 0..65, interior cols 2..65
NPIX = H * W            # 4096
KSL = HID * NPIX // 8   # 16384 per-core FC1 K-slice
N_CORES = 8

TRACE = False           # test.py flips this for profiled runs
_CACHE = {}


def _build_nc():
    nc = bacc.Bacc("TRN2", target_bir_lowering=False, debug=False,
                   num_devices=N_CORES)

    # ---- I/O -------------------------------------------------------------
    xp_d = nc.dram_tensor("xp", [T, C, PH, PW], F16, kind="ExternalInput")
    w0_d = nc.dram_tensor("w0", [105, 3 * G4], F16, kind="ExternalInput")
    w1a_d = nc.dram_tensor("w1a", [128, 3 * G4], F16, kind="ExternalInput")
    w1b_d = nc.dram_tensor("w1b", [64, 3 * G4], F16, kind="ExternalInput")
    wd_d = nc.dram_tensor("wd", [105, 3 * G4], F16, kind="ExternalInput")
    b0_d = nc.dram_tensor("b0", [G4, 1], F32, kind="ExternalInput")
    b1_d = nc.dram_tensor("b1", [G4, 1], F32, kind="ExternalInput")
    bd_d = nc.dram_tensor("bd", [G4, 1], F32, kind="ExternalInput")
    fw_d = nc.dram_tensor("fw", [128, 128 * 256], F16, kind="ExternalInput")
    fb_d = nc.dram_tensor("fb", [128, 2], F32, kind="ExternalInput")
    w2_d = nc.dram_tensor("w2", [128, 2 * 97], F16, kind="ExternalInput")
    b2_d = nc.dram_tensor("b2", [97, 1], F32, kind="ExternalInput")
    out_d = nc.dram_tensor("out", [97, 1], F32, kind="ExternalOutput")

    with tile.TileContext(nc) as tc:
        with (
            tc.tile_pool(name="state", bufs=1) as state,
            tc.tile_pool(name="const", bufs=1) as const,
            tc.tile_pool(name="sgate", bufs=2) as sgate,
            tc.tile_pool(name="scr", bufs=2) as scr,
            tc.tile_pool(name="psum", bufs=8, space="PSUM") as psum,
            tc.tile_pool(name="dram", bufs=1, space="DRAM") as dram,
        ):
            # ---- persistent SBUF state ----------------------------------
            A = state.tile([105, PH, PW], F16)    # L0 rhs (h0 + x), 3 ky-groups
            Ba = state.tile([128, PH, PW], F16)   # L1 rhs ky=-1,0
            Bb = state.tile([64, PH, PW], F16)    # L1 rhs ky=+1
            # c-state lives on partitions 32-63 so TT ops pair with S[32:64]
            cst0 = state.tile([64, NPIX], F16)
            cst1 = state.tile([64, NPIX], F16)
            hdc = state.tile([HID, NPIX], F16)    # decoder h (feat)

            # ---- constants ----------------------------------------------
            w0 = const.tile([105, 3 * G4], F16)
            w1a = const.tile([128, 3 * G4], F16)
            w1b = const.tile([64, 3 * G4], F16)
            wd = const.tile([105, 3 * G4], F16)
            b0 = const.tile([G4, 1], F32)
            b1 = const.tile([G4, 1], F32)
            bd = const.tile([G4, 1], F32)
            fw = const.tile([128, 128 * 256], F16)
            fb = const.tile([128, 2], F32)
            w2 = const.tile([128, 2 * 97], F16)
            b2 = const.tile([97, 1], F32)
            ft = const.tile([128, 8, 128], F16)   # A2A result, FC1 lhsT tiles

            for dst, src in ((w0, w0_d), (w1a, w1a_d), (w1b, w1b_d),
                             (wd, wd_d), (b0, b0_d), (b1, b1_d), (bd, bd_d),
                             (fb, fb_d), (w2, w2_d), (b2, b2_d)):
                nc.sync.dma_start(out=dst[:], in_=src[:])
            # big fc1 weight: split across queues
            for i in range(8):
                sl = slice(i * 4096, (i + 1) * 4096)
                nc.sync.dma_start(out=fw[:, sl], in_=fw_d[:, sl])

            # zero-init state (h=0, c=0, halos=0)
            nc.gpsimd.memset(A[:], 0.0)
            nc.gpsimd.memset(Ba[:], 0.0)
            nc.gpsimd.memset(Bb[:], 0.0)
            nc.vector.memset(cst0[:], 0.0)
            nc.vector.memset(cst1[:], 0.0)

            # ---- DRAM bounce buffers for collectives --------------------
            a2a_in = dram.tile([HID, NPIX], F16)
            a2a_out = dram.tile([8, 128, 128], F16)
            z1part = dram.tile([8, 256], F32)
            z1red = dram.tile([256], F32)

            KXS = (-1, 0, 1)

            def conv_layer(srcs, bias, S):
                """srcs: list of (tile, K, weights); accumulate 3 kx passes
                each into 8 row-tiles of PSUM, then sigmoid -> S."""
                pst = [psum.tile([G4, 512], F32, tag="z", name=f"pz{_}") for _ in range(8)]
                npass = len(srcs) * 3
                ip = 0
                for buf, K, wt in srcs:
                    for kxi, kx in enumerate(KXS):
                        lhs = wt[:, kxi * G4:(kxi + 1) * G4]
                        for rt in range(8):
                            rhs = buf[0:K, 8 * rt + 1:8 * rt + 9,
                                      2 + kx:66 + kx]
                            nc.tensor.matmul(pst[rt][:], lhs, rhs,
                                             start=(ip == 0),
                                             stop=(ip == npass - 1))
                        ip += 1
                for rt in range(8):
                    nc.scalar.activation(out=S[:, rt * 512:(rt + 1) * 512],
                                         in_=pst[rt][:], func=AF.Sigmoid,
                                         bias=bias[:, 0:1], scale=1.0)

            def pointwise(S, cst, hdst):
                """LSTM cell state update + h writes (two column halves).
                TT inputs must share a base partition, so scratch tensors
                are placed at the base of the gate they pair with."""
                for hv in range(2):
                    sl = slice(hv * 2048, (hv + 1) * 2048)
                    # tg = 2*sigmoid(2g) - 1, re-based to partitions 0-31
                    tgt = scr.tile([32, 2048], F16, tag="tgt")
                    nc.vector.tensor_scalar(
                        out=tgt[:], in0=S[96:128, sl],
                        scalar1=2.0, scalar2=-1.0, op0=OP.mult, op1=OP.add)
                    uv = scr.tile([32, 2, 2048], F16, tag="uv")
                    nc.vector.tensor_mul(uv[:, 0, :], S[0:32, sl], tgt[:])
                    nc.vector.tensor_mul(uv[:, 1, :], S[32:64, sl],
                                         cst[32:64, sl])
                    nc.vector.tensor_add(cst[32:64, sl], uv[:, 0, :],
                                         uv[:, 1, :])
                    tht = scr.tile([96, 2048], F16, tag="tht")
                    nc.scalar.activation(out=tht[64:96, :],
                                         in_=cst[32:64, sl], func=AF.Tanh)
                    if hdst is hdc:
                        dst = hdc[:, sl]
                    else:
                        buf, p0 = hdst
                        dst = buf[p0:p0 + 32, 1 + 32 * hv:33 + 32 * hv, 2:66]
                    nc.vector.tensor_mul(dst, S[64:96, sl], tht[64:96, :])

            def shift_copies(dsts, src):
                """src: (buf, p0) base-group h [32, PH, PW]; dsts: list of
                (buf, p0, ky)."""
                sbuf, sp = src
                for buf, p0, ky in dsts:
                    if ky == 0:
                        nc.sync.dma_start(out=buf[p0:p0 + 32, :, :],
                                          in_=sbuf[sp:sp + 32, :, :])
                    elif ky == -1:
                        nc.sync.dma_start(out=buf[p0:p0 + 32, 1:PH, :],
                                          in_=sbuf[sp:sp + 32, 0:PH - 1, :])
                    else:
                        nc.sync.dma_start(out=buf[p0:p0 + 32, 0:PH - 1, :],
                                          in_=sbuf[sp:sp + 32, 1:PH, :])

            # ================= recurrent steps ===========================
            for t in range(T):
                # x_t into A's 3 ky-groups (ky=0 @32, ky=-1 @67, ky=+1 @102)
                nc.sync.dma_start(out=A[32:35, :, :], in_=xp_d[t])
                nc.sync.dma_start(out=A[67:70, 1:PH, :],
                                  in_=xp_d[t, :, 0:PH - 1, :])
                nc.sync.dma_start(out=A[102:105, 0:PH - 1, :],
                                  in_=xp_d[t, :, 1:PH, :])

                S0 = sgate.tile([G4, NPIX], F16, tag="S")
                conv_layer([(A, 105, w0)], b0, S0)
                pointwise(S0, cst0, (A, 0))
                # replicate h0 into shifted groups (A) and layer-1 inputs
                shift_copies([(A, 35, -1), (A, 70, 1),
                              (Ba, 64, 0), (Ba, 0, -1), (Bb, 0, 1)],
                             (A, 0))

                S1 = sgate.tile([G4, NPIX], F16, tag="S")
                conv_layer([(Ba, 128, w1a), (Bb, 64, w1b)], b1, S1)
                pointwise(S1, cst1, (Ba, 96))
                shift_copies([(Ba, 32, -1), (Bb, 32, 1)], (Ba, 96))

            # ================= decoder step ==============================
            shift_copies([(A, 0, 0), (A, 35, -1), (A, 70, 1)], (Ba, 96))
            Sd = sgate.tile([G4, NPIX], F16, tag="S")
            conv_layer([(A, 105, wd)], bd, Sd)
            pointwise(Sd, cst1, hdc)

            # ================= FC head ===================================
            nc.sync.dma_start(out=a2a_in[:], in_=hdc[:])
            nc.gpsimd.collective_compute(
                "AllToAll", OP.bypass,
                replica_groups=[list(range(N_CORES))],
                ins=[a2a_in[:].opt()], outs=[a2a_out[:].opt()])
            # transposed load with K-index q = p*128 + k2:
            # ft[p, m, k2] = a2a_out[m, p, k2] -- contiguous 128-elem runs
            nc.sync.dma_start(
                out=ft[:],
                in_=a2a_out[:].rearrange("m p k -> p m k"))

            psz = psum.tile([8, 256], F32, tag="z")
            for k2 in range(128):
                nc.tensor.matmul(psz[:], ft[:, :, k2],
                                 fw[:, k2 * 256:(k2 + 1) * 256],
                                 start=(k2 == 0), stop=(k2 == 127))
            z1s = scr.tile([8, 256], F32, tag="z1")
            nc.vector.tensor_copy(z1s[:], psz[:])
            nc.sync.dma_start(out=z1part[:], in_=z1s[:])
            nc.gpsimd.collective_compute(
                "ReduceScatter", OP.add,
                replica_groups=[list(range(N_CORES))],
                ins=[z1part[:].opt()], outs=[z1red[:].opt()])

            zr = scr.tile([128, 2], F32, tag="zr")
            nc.sync.dma_start(out=zr[:],
                              in_=z1red[:].rearrange("(j p) -> p j", p=128))
            zrb = scr.tile([128, 2], F32, tag="zrb")
            nc.vector.tensor_add(zrb[:], zr[:], fb[:])
            h256 = scr.tile([128, 2], F16, tag="h256")
            nc.vector.tensor_scalar_max(h256[:], zrb[:], 0.0)

            ps2 = psum.tile([97, 1], F32, tag="z")
            for j in range(2):
                nc.tensor.matmul(ps2[:], w2[:, j * 97:(j + 1) * 97],
                                 h256[:, j:j + 1],
                                 start=(j == 0), stop=(j == 1))
            outs = scr.tile([97, 1], F32, tag="outs")
            nc.vector.tensor_add(outs[:], ps2[:], b2[:])
            nc.sync.dma_start(out=out_d[:], in_=outs[:])

    nc.compile()
    return nc


def _prep_inputs(x, Wenc0, benc0, Wenc1, benc1, Wdec, bdec,
                 fc1_w, fc1_b, fc2_w, fc2_b):
    """Host-side: pad/reorder/cast everything into device layouts."""
    f16 = np.float16

    def conv_w(Wk, reorder_x):
        # Wk [128, Cin, 3, 3] -> per-kx [ngrp*ch, 128] with ky stacked on
        # partitions; gate-g output channels pre-scaled x2.
        Wk = np.asarray(Wk, np.float32).copy()
        Wk[96:128] *= 2.0
        if reorder_x:  # [x(3), h(32)] -> [h(32), x(3)]
            Wk = np.concatenate([Wk[:, 3:], Wk[:, :3]], axis=1)
        cin = Wk.shape[1]
        out = np.zeros((3 * cin, 3 * G4), np.float32)
        for g, dy in enumerate((1, 0, 2)):   # group order ky = 0, -1, +1
            for kxi in range(3):
                # [cin, 128]
                out[g * cin:(g + 1) * cin, kxi * G4:(kxi + 1) * G4] = \
                    Wk[:, :, dy, kxi].T
        return out.astype(f16)

    def bias_v(b):
        b = np.asarray(b, np.float32).copy()
        b[96:128] *= 2.0
        return b.reshape(G4, 1)

    w0_full = conv_w(Wenc0, True)       # [105, 384]
    wd_full = conv_w(Wdec, True)
    w1_full = conv_w(Wenc1, False)      # [192, 384]; groups ky = 0, -1, +1
    # Ba's partition groups are ky=-1 @0-63, ky=0 @64-127
    w1a = np.ascontiguousarray(
        np.concatenate([w1_full[64:128], w1_full[0:64]], axis=0))
    w1b = np.ascontiguousarray(w1_full[128:192])

    xpad = np.zeros((B, T, C, PH, PW), f16)
    xpad[:, :, :, 1:65, 2:66] = np.asarray(x, np.float32)

    fc1_w = np.asarray(fc1_w, np.float32)
    fb = np.asarray(fc1_b, np.float32).reshape(2, 128).T.copy()  # [128, 2]
    w2 = np.asarray(fc2_w, np.float32).T.reshape(2, 128, 97)
    w2 = np.ascontiguousarray(w2.transpose(1, 0, 2)).reshape(128, 2 * 97)
    b2 = np.asarray(fc2_b, np.float32).reshape(97, 1)

    in_maps = []
    for k in range(N_CORES):
        w1k = fc1_w[:, k * KSL:(k + 1) * KSL].T            # [16384, 256]
        # K-index q = p*128 + k2  ->  fw[p, k2, n] = w1k[p*128 + k2, n]
        fwk = w1k.reshape(128, 128 * 256)
        in_maps.append({
            "xp": np.ascontiguousarray(xpad[k]),
            "w0": w0_full, "w1a": w1a.astype(f16), "w1b": w1b.astype(f16),
            "wd": wd_full,
            "b0": bias_v(benc0), "b1": bias_v(benc1), "bd": bias_v(bdec),
            "fw": fwk.astype(f16), "fb": fb,
            "w2": w2.astype(f16), "b2": b2,
        })
    return in_maps


def kernel(**inputs):
    if "nc" not in _CACHE:
        _CACHE["nc"] = _build_nc()
    nc = _CACHE["nc"]
    in_maps = _prep_inputs(**inputs)
    res = run_bass_kernel_spmd(nc, in_maps, core_ids=list(range(N_CORES)),
                               trace=TRACE)
    _CACHE["last_result"] = res
    out = np.stack([res.results[k]["out"][:, 0] for k in range(N_CORES)])
    return out.astype(np.float32)


# revision 17
# speedup vs baseline: 1.1119x; 1.1119x over previous
"""Trainium2 Bass kernel for nn_BaltNet (2-layer ConvLSTM + decoder + MLP head).

Sharding: data-parallel over batch B=8 (one sample per NeuronCore) for the
recurrent conv part; FC1's [131072, 256] contraction is K-sharded 8 ways
(AllToAll of the decoder features, per-core partial matmul, ReduceScatter).

Layout notes
------------
Conv is computed as matmuls over a zero-padded spatial layout [C, 66, 68]
(1-row halo top/bottom, cols 2..65 interior) so every 3x3 tap is a pure
free-dim offset.  The three vertical taps (ky) are packed into the matmul
contraction dim by keeping row-shifted copies of the input stacked on
partitions; the three horizontal taps (kx) are separate accumulating matmul
passes with shifted column windows.

  A  [105, 66, 68]: layer-0 rhs, 3 groups of (h0[32] + x[3]) at ky=0,-1,+1
      (base group first: engine writes need 32-aligned partition starts)
  Ba [128, 66, 68]: layer-1 rhs, groups (h0+h1)[64] at ky=-1 (p0-63), ky=0
  Bb [ 64, 66, 68]: layer-1 rhs, group  (h0+h1)[64] at ky=+1

Gates: z = [i f o g] on 128 partitions; g-gate weights/bias pre-scaled x2 so
tanh(g) = 2*sigmoid(2g) - 1 and one Sigmoid covers all 128 partitions.
Everything 16-bit is fp16 (verified ~1.2e-3 end-to-end vs fp32 reference).
"""

import os
import sys

for _p in ("/opt/trn_rl_repo",):
    if _p not in sys.path and os.path.isdir(_p):
        sys.path.insert(0, _p)

import numpy as np

import concourse.bass as bass
import concourse.mybir as mybir
import concourse.tile as tile
from concourse import bacc
from concourse.bass_utils import run_bass_kernel_spmd

F16 = mybir.dt.float16
F32 = mybir.dt.float32
AF = mybir.ActivationFunctionType
OP = mybir.AluOpType

B, T, C, HID, H, W = 8, 24, 3, 32, 64, 64
G4 = 4 * HID            # 128 gate channels
PH, PW = H + 2, W + 4   # padded spatial: rows 0..65, interior cols 2..65
NPIX = H * W            # 4096
KSL = HID * NPIX // 8   # 16384 per-core FC1 K-slice
N_CORES = 8

TRACE = False           # test.py flips this for profiled runs
_CACHE = {}


def _build_nc():
    nc = bacc.Bacc("TRN2", target_bir_lowering=False, debug=False,
                   num_devices=N_CORES)

    # ---- I/O -------------------------------------------------------------
    xp_d = nc.dram_tensor("xp", [T, C, PH, PW], F16, kind="ExternalInput")
    w0_d = nc.dram_tensor("w0", [105, 3 * G4], F16, kind="ExternalInput")
    w1a_d = nc.dram_tensor("w1a", [128, 3 * G4], F16, kind="ExternalInput")
    w1b_d = nc.dram_tensor("w1b", [64, 3 * G4], F16, kind="ExternalInput")
    wd_d = nc.dram_tensor("wd", [105, 3 * G4], F16, kind="ExternalInput")
    b0_d = nc.dram_tensor("b0", [G4, 1], F32, kind="ExternalInput")
    b1_d = nc.dram_tensor("b1", [G4, 1], F32, kind="ExternalInput")
    bd_d = nc.dram_tensor("bd", [G4, 1], F32, kind="ExternalInput")
    fw_d = nc.dram_tensor("fw", [128, 128 * 256], F16, kind="ExternalInput")
    fb_d = nc.dram_tensor("fb", [128, 2], F32, kind="ExternalInput")
    w2_d = nc.dram_tensor("w2", [128, 2 * 97], F16, kind="ExternalInput")
    b2_d = nc.dram_tensor("b2", [97, 1], F32, kind="ExternalInput")
    out_d = nc.dram_tensor("out", [97, 1], F32, kind="ExternalOutput")

    with tile.TileContext(nc) as tc:
        with (
            tc.tile_pool(name="state", bufs=1) as state,
            tc.tile_pool(name="const", bufs=1) as const,
            tc.tile_pool(name="sgate", bufs=2) as sgate,
            tc.tile_pool(name="scr", bufs=2) as scr,
            tc.tile_pool(name="psum", bufs=4, space="PSUM") as psum,
            tc.tile_pool(name="dram", bufs=1, space="DRAM") as dram,
        ):
            # ---- persistent SBUF state ----------------------------------
            A = state.tile([105, PH, PW], F16)    # L0 rhs (h0 + x), 3 ky-groups
            Ba = state.tile([128, PH, PW], F16)   # L1 rhs ky=-1,0
            Bb = state.tile([64, PH, PW], F16)    # L1 rhs ky=+1
            # c-state lives on partitions 32-63 so TT ops pair with S[32:64]
            cst0 = state.tile([64, NPIX], F16)
            cst1 = state.tile([64, NPIX], F16)
            hdc = state.tile([HID, NPIX], F16)    # decoder h (feat)

            # ---- constants ----------------------------------------------
            w0 = const.tile([105, 3 * G4], F16)
            w1a = const.tile([128, 3 * G4], F16)
            w1b = const.tile([64, 3 * G4], F16)
            wd = const.tile([105, 3 * G4], F16)
            b0 = const.tile([G4, 1], F32)
            b1 = const.tile([G4, 1], F32)
            bd = const.tile([G4, 1], F32)
            fw = const.tile([128, 128 * 256], F16)
            fb = const.tile([128, 2], F32)
            w2 = const.tile([128, 2 * 97], F16)
            b2 = const.tile([97, 1], F32)
            ft = const.tile([128, 8, 128], F16)   # A2A result, FC1 lhsT tiles

            for dst, src in ((w0, w0_d), (w1a, w1a_d), (w1b, w1b_d),
                             (wd, wd_d), (b0, b0_d), (b1, b1_d), (bd, bd_d),
                             (fb, fb_d), (w2, w2_d), (b2, b2_d)):
                nc.sync.dma_start(out=dst[:], in_=src[:])
            # big fc1 weight: split across queues
            for i in range(8):
                sl = slice(i * 4096, (i + 1) * 4096)
                nc.sync.dma_start(out=fw[:, sl], in_=fw_d[:, sl])

            # zero-init state (h=0, c=0, halos=0)
            nc.gpsimd.memset(A[:], 0.0)
            nc.gpsimd.memset(Ba[:], 0.0)
            nc.gpsimd.memset(Bb[:], 0.0)
            nc.vector.memset(cst0[:], 0.0)
            nc.vector.memset(cst1[:], 0.0)

            # ---- DRAM bounce buffers for collectives --------------------
            a2a_in = dram.tile([HID, NPIX], F16)
            a2a_out = dram.tile([8, 128, 128], F16)
            z1part = dram.tile([8, 256], F32)
            z1red = dram.tile([256], F32)

            KXS = (-1, 0, 1)

            def conv_layer(srcs, bias, S):
                """srcs: list of (tile, K, weights); accumulate 3 kx passes
                each into 4 row-tiles of PSUM (2 banks apiece), then
                sigmoid -> S."""
                pst = [psum.tile([G4, 1024], F32, tag="z", name=f"pz{_}")
                       for _ in range(4)]
                npass = len(srcs) * 3
                ip = 0
                for buf, K, wt in srcs:
                    for kxi, kx in enumerate(KXS):
                        lhs = wt[:, kxi * G4:(kxi + 1) * G4]
                        for rt in range(4):
                            for h in range(2):
                                r0 = 16 * rt + 8 * h
                                rhs = buf[0:K, r0 + 1:r0 + 9, 2 + kx:66 + kx]
                                nc.tensor.matmul(
                                    pst[rt][:, 512 * h:512 * h + 512],
                                    lhs, rhs, start=(ip == 0),
                                    stop=(ip == npass - 1))
                        ip += 1
                for rt in range(4):
                    nc.scalar.activation(out=S[:, rt * 1024:(rt + 1) * 1024],
                                         in_=pst[rt][:], func=AF.Sigmoid,
                                         bias=bias[:, 0:1], scale=1.0)

            def pointwise(S, cst, hdst):
                """LSTM cell state update + h writes (two column halves).
                TT inputs must share a base partition, so scratch tensors
                are placed at the base of the gate they pair with."""
                for hv in range(2):
                    sl = slice(hv * 2048, (hv + 1) * 2048)
                    # tg = 2*sigmoid(2g) - 1, re-based to partitions 0-31
                    tgt = scr.tile([32, 2048], F16, tag="tgt")
                    nc.vector.tensor_scalar(
                        out=tgt[:], in0=S[96:128, sl],
                        scalar1=2.0, scalar2=-1.0, op0=OP.mult, op1=OP.add)
                    uv = scr.tile([32, 2, 2048], F16, tag="uv")
                    nc.vector.tensor_mul(uv[:, 0, :], S[0:32, sl], tgt[:])
                    nc.vector.tensor_mul(uv[:, 1, :], S[32:64, sl],
                                         cst[32:64, sl])
                    nc.vector.tensor_add(cst[32:64, sl], uv[:, 0, :],
                                         uv[:, 1, :])
                    tht = scr.tile([96, 2048], F16, tag="tht")
                    nc.scalar.activation(out=tht[64:96, :],
                                         in_=cst[32:64, sl], func=AF.Tanh)
                    if hdst is hdc:
                        dst = hdc[:, sl]
                    else:
                        buf, p0 = hdst
                        dst = buf[p0:p0 + 32, 1 + 32 * hv:33 + 32 * hv, 2:66]
                    nc.vector.tensor_mul(dst, S[64:96, sl], tht[64:96, :])

            def shift_copies(dsts, src):
                """src: (buf, p0) base-group h [32, PH, PW]; dsts: list of
                (buf, p0, ky)."""
                sbuf, sp = src
                for buf, p0, ky in dsts:
                    if ky == 0:
                        nc.sync.dma_start(out=buf[p0:p0 + 32, :, :],
                                          in_=sbuf[sp:sp + 32, :, :])
                    elif ky == -1:
                        nc.sync.dma_start(out=buf[p0:p0 + 32, 1:PH, :],
                                          in_=sbuf[sp:sp + 32, 0:PH - 1, :])
                    else:
                        nc.sync.dma_start(out=buf[p0:p0 + 32, 0:PH - 1, :],
                                          in_=sbuf[sp:sp + 32, 1:PH, :])

            # ================= recurrent steps ===========================
            # Layer 1 runs one step behind layer 0 so the PE alternates
            # between the two layers' matmul bursts with no pointwise gap:
            # L1(t-1)'s inputs (h0(t-1), h1(t-2)) are ready before L0(t)
            # even starts.  The h0(t) -> Ba/Bb copies are emitted AFTER
            # L1(t-1) so Tile's program-order dependency keeps them WAR.
            def l1_step():
                S1 = sgate.tile([G4, NPIX], F16, tag="S", name="S1")
                conv_layer([(Ba, 128, w1a), (Bb, 64, w1b)], b1, S1)
                pointwise(S1, cst1, (Ba, 96))
                shift_copies([(Ba, 32, -1), (Bb, 32, 1)], (Ba, 96))

            for t in range(T):
                # x_t into A's 3 ky-groups (ky=0 @32, ky=-1 @67, ky=+1 @102)
                nc.sync.dma_start(out=A[32:35, :, :], in_=xp_d[t])
                nc.sync.dma_start(out=A[67:70, 1:PH, :],
                                  in_=xp_d[t, :, 0:PH - 1, :])
                nc.sync.dma_start(out=A[102:105, 0:PH - 1, :],
                                  in_=xp_d[t, :, 1:PH, :])

                S0 = sgate.tile([G4, NPIX], F16, tag="S", name="S0")
                conv_layer([(A, 105, w0)], b0, S0)
                pointwise(S0, cst0, (A, 0))
                # h0(t) shifted copies within A (next L0 step's rhs)
                shift_copies([(A, 35, -1), (A, 70, 1)], (A, 0))

                if t > 0:
                    l1_step()          # L1(t-1)
                # now h0(t) may overwrite L1's rhs state
                shift_copies([(Ba, 64, 0), (Ba, 0, -1), (Bb, 0, 1)], (A, 0))

            l1_step()                  # L1(T-1)

            # ================= decoder step ==============================
            shift_copies([(A, 0, 0), (A, 35, -1), (A, 70, 1)], (Ba, 96))
            Sd = sgate.tile([G4, NPIX], F16, tag="S")
            conv_layer([(A, 105, wd)], bd, Sd)
            pointwise(Sd, cst1, hdc)

            # ================= FC head ===================================
            nc.sync.dma_start(out=a2a_in[:], in_=hdc[:])
            nc.gpsimd.collective_compute(
                "AllToAll", OP.bypass,
                replica_groups=[list(range(N_CORES))],
                ins=[a2a_in[:].opt()], outs=[a2a_out[:].opt()])
            # transposed load with K-index q = p*128 + k2:
            # ft[p, m, k2] = a2a_out[m, p, k2] -- contiguous 128-elem runs
            nc.sync.dma_start(
                out=ft[:],
                in_=a2a_out[:].rearrange("m p k -> p m k"))

            psz = psum.tile([8, 256], F32, tag="z")
            for k2 in range(128):
                nc.tensor.matmul(psz[:], ft[:, :, k2],
                                 fw[:, k2 * 256:(k2 + 1) * 256],
                                 start=(k2 == 0), stop=(k2 == 127))
            z1s = scr.tile([8, 256], F32, tag="z1")
            nc.vector.tensor_copy(z1s[:], psz[:])
            nc.sync.dma_start(out=z1part[:], in_=z1s[:])
            nc.gpsimd.collective_compute(
                "ReduceScatter", OP.add,
                replica_groups=[list(range(N_CORES))],
                ins=[z1part[:].opt()], outs=[z1red[:].opt()])

            zr = scr.tile([128, 2], F32, tag="zr")
            nc.sync.dma_start(out=zr[:],
                              in_=z1red[:].rearrange("(j p) -> p j", p=128))
            zrb = scr.tile([128, 2], F32, tag="zrb")
            nc.vector.tensor_add(zrb[:], zr[:], fb[:])
            h256 = scr.tile([128, 2], F16, tag="h256")
            nc.vector.tensor_scalar_max(h256[:], zrb[:], 0.0)

            ps2 = psum.tile([97, 1], F32, tag="z")
            for j in range(2):
                nc.tensor.matmul(ps2[:], w2[:, j * 97:(j + 1) * 97],
                                 h256[:, j:j + 1],
                                 start=(j == 0), stop=(j == 1))
            outs = scr.tile([97, 1], F32, tag="outs")
            nc.vector.tensor_add(outs[:], ps2[:], b2[:])
            nc.sync.dma_start(out=out_d[:], in_=outs[:])

    nc.compile()
    return nc


def _prep_inputs(x, Wenc0, benc0, Wenc1, benc1, Wdec, bdec,
                 fc1_w, fc1_b, fc2_w, fc2_b):
    """Host-side: pad/reorder/cast everything into device layouts."""
    f16 = np.float16

    def conv_w(Wk, reorder_x):
        # Wk [128, Cin, 3, 3] -> per-kx [ngrp*ch, 128] with ky stacked on
        # partitions; gate-g output channels pre-scaled x2.
        Wk = np.asarray(Wk, np.float32).copy()
        Wk[96:128] *= 2.0
        if reorder_x:  # [x(3), h(32)] -> [h(32), x(3)]
            Wk = np.concatenate([Wk[:, 3:], Wk[:, :3]], axis=1)
        cin = Wk.shape[1]
        out = np.zeros((3 * cin, 3 * G4), np.float32)
        for g, dy in enumerate((1, 0, 2)):   # group order ky = 0, -1, +1
            for kxi in range(3):
                # [cin, 128]
                out[g * cin:(g + 1) * cin, kxi * G4:(kxi + 1) * G4] = \
                    Wk[:, :, dy, kxi].T
        return out.astype(f16)

    def bias_v(b):
        b = np.asarray(b, np.float32).copy()
        b[96:128] *= 2.0
        return b.reshape(G4, 1)

    w0_full = conv_w(Wenc0, True)       # [105, 384]
    wd_full = conv_w(Wdec, True)
    w1_full = conv_w(Wenc1, False)      # [192, 384]; groups ky = 0, -1, +1
    # Ba's partition groups are ky=-1 @0-63, ky=0 @64-127
    w1a = np.ascontiguousarray(
        np.concatenate([w1_full[64:128], w1_full[0:64]], axis=0))
    w1b = np.ascontiguousarray(w1_full[128:192])

    xpad = np.zeros((B, T, C, PH, PW), f16)
    xpad[:, :, :, 1:65, 2:66] = np.asarray(x, np.float32)

    fc1_w = np.asarray(fc1_w, np.float32)
    fb = np.asarray(fc1_b, np.float32).reshape(2, 128).T.copy()  # [128, 2]
    w2 = np.asarray(fc2_w, np.float32).T.reshape(2, 128, 97)
    w2 = np.ascontiguousarray(w2.transpose(1, 0, 2)).reshape(128, 2 * 97)
    b2 = np.asarray(fc2_b, np.float32).reshape(97, 1)

    in_maps = []
    for k in range(N_CORES):
        w1k = fc1_w[:, k * KSL:(k + 1) * KSL].T            # [16384, 256]
        # K-index q = p*128 + k2  ->  fw[p, k2, n] = w1k[p*128 + k2, n]
        fwk = w1k.reshape(128, 128 * 256)
        in_maps.append({
            "xp": np.ascontiguousarray(xpad[k]),
            "w0": w0_full, "w1a": w1a.astype(f16), "w1b": w1b.astype(f16),
            "wd": wd_full,
            "b0": bias_v(benc0), "b1": bias_v(benc1), "bd": bias_v(bdec),
            "fw": fwk.astype(f16), "fb": fb,
            "w2": w2.astype(f16), "b2": b2,
        })
    return in_maps


def kernel(**inputs):
    if "nc" not in _CACHE:
        _CACHE["nc"] = _build_nc()
    nc = _CACHE["nc"]
    in_maps = _prep_inputs(**inputs)
    res = run_bass_kernel_spmd(nc, in_maps, core_ids=list(range(N_CORES)),
                               trace=TRACE)
    _CACHE["last_result"] = res
    out = np.stack([res.results[k]["out"][:, 0] for k in range(N_CORES)])
    return out.astype(np.float32)


# revision 20
# speedup vs baseline: 1.4814x; 1.3323x over previous
"""Trainium2 Bass kernel for nn_BaltNet (2-layer ConvLSTM + decoder + MLP head).

Sharding: data-parallel over batch B=8 (one sample per NeuronCore) for the
recurrent conv part; FC1's [131072, 256] contraction is K-sharded 8 ways
(AllToAll of the decoder features, per-core partial matmul, ReduceScatter).

Layout notes
------------
Conv is computed as matmuls over a zero-padded spatial layout [C, 66, 68]
(1-row halo top/bottom, cols 2..65 interior) so every 3x3 tap is a pure
free-dim offset.  The three vertical taps (ky) are packed into the matmul
contraction dim by keeping row-shifted copies of the input stacked on
partitions; the three horizontal taps (kx) are separate accumulating matmul
passes with shifted column windows.

  A  [105, 66, 68]: layer-0 rhs, 3 groups of (h0[32] + x[3]) at ky=0,-1,+1
      (base group first: engine writes need 32-aligned partition starts)
  Ba [128, 66, 68]: layer-1 rhs, groups (h0+h1)[64] at ky=-1 (p0-63), ky=0
  Bb [ 64, 66, 68]: layer-1 rhs, group  (h0+h1)[64] at ky=+1

Gates: z = [i f o g] on 128 partitions; g-gate weights/bias pre-scaled x2 so
tanh(g) = 2*sigmoid(2g) - 1 and one Sigmoid covers all 128 partitions.
Everything 16-bit is fp16 (verified ~1.2e-3 end-to-end vs fp32 reference).
"""

import os
import sys

for _p in ("/opt/trn_rl_repo",):
    if _p not in sys.path and os.path.isdir(_p):
        sys.path.insert(0, _p)

import numpy as np

import concourse.bass as bass
import concourse.mybir as mybir
import concourse.tile as tile
from concourse import bacc
from concourse.bass_utils import run_bass_kernel_spmd

F16 = mybir.dt.float16
F32 = mybir.dt.float32
AF = mybir.ActivationFunctionType
OP = mybir.AluOpType

B, T, C, HID, H, W = 8, 24, 3, 32, 64, 64
G4 = 4 * HID            # 128 gate channels
PH, PW = H + 2, W + 4   # padded spatial: rows 0..65, interior cols 2..65
NPIX = H * W            # 4096
KSL = HID * NPIX // 8   # 16384 per-core FC1 K-slice
N_CORES = 8

TRACE = False           # test.py flips this for profiled runs
_CACHE = {}


def _build_nc():
    nc = bacc.Bacc("TRN2", target_bir_lowering=False, debug=False,
                   num_devices=N_CORES)

    # ---- I/O -------------------------------------------------------------
    xp_d = nc.dram_tensor("xp", [T, C, PH, PW], F16, kind="ExternalInput")
    w0_d = nc.dram_tensor("w0", [105, 3 * G4], F16, kind="ExternalInput")
    w1a_d = nc.dram_tensor("w1a", [128, 3 * G4], F16, kind="ExternalInput")
    w1b_d = nc.dram_tensor("w1b", [64, 3 * G4], F16, kind="ExternalInput")
    wd_d = nc.dram_tensor("wd", [105, 3 * G4], F16, kind="ExternalInput")
    b0_d = nc.dram_tensor("b0", [G4, 1], F32, kind="ExternalInput")
    b1_d = nc.dram_tensor("b1", [G4, 1], F32, kind="ExternalInput")
    bd_d = nc.dram_tensor("bd", [G4, 1], F32, kind="ExternalInput")
    fw_d = nc.dram_tensor("fw", [128, 128 * 256], F16, kind="ExternalInput")
    fb_d = nc.dram_tensor("fb", [128, 2], F32, kind="ExternalInput")
    w2_d = nc.dram_tensor("w2", [128, 2 * 97], F16, kind="ExternalInput")
    b2_d = nc.dram_tensor("b2", [97, 1], F32, kind="ExternalInput")
    out_d = nc.dram_tensor("out", [97, 1], F32, kind="ExternalOutput")

    with tile.TileContext(nc) as tc:
        with (
            tc.tile_pool(name="state", bufs=1) as state,
            tc.tile_pool(name="const", bufs=1) as const,
            tc.tile_pool(name="sgate", bufs=3) as sgate,
            tc.tile_pool(name="scr", bufs=3) as scr,
            tc.tile_pool(name="psum", bufs=4, space="PSUM") as psum,
            tc.tile_pool(name="dram", bufs=1, space="DRAM") as dram,
        ):
            # ---- persistent SBUF state ----------------------------------
            A = state.tile([105, PH, PW], F16)    # L0 rhs (h0 + x), 3 ky-groups
            Ba = state.tile([128, PH, PW], F16)   # L1 rhs ky=-1,0
            Bb = state.tile([64, PH, PW], F16)    # L1 rhs ky=+1
            # c-state lives on partitions 32-63 so TT ops pair with S[32:64]
            cst0 = state.tile([64, NPIX], F16)
            cst1 = state.tile([64, NPIX], F16)
            hdc = state.tile([HID, NPIX], F16)    # decoder h (feat)

            # ---- constants ----------------------------------------------
            w0 = const.tile([105, 3 * G4], F16)
            w1a = const.tile([128, 3 * G4], F16)
            w1b = const.tile([64, 3 * G4], F16)
            wd = const.tile([105, 3 * G4], F16)
            b0 = const.tile([G4, 1], F32)
            b1 = const.tile([G4, 1], F32)
            bd = const.tile([G4, 1], F32)
            fw = const.tile([128, 128 * 256], F16)
            fb = const.tile([128, 2], F32)
            w2 = const.tile([128, 2 * 97], F16)
            b2 = const.tile([97, 1], F32)
            ft = const.tile([128, 8, 128], F16)   # A2A result, FC1 lhsT tiles

            for dst, src in ((w0, w0_d), (w1a, w1a_d), (w1b, w1b_d),
                             (wd, wd_d), (b0, b0_d), (b1, b1_d), (bd, bd_d),
                             (fb, fb_d), (w2, w2_d), (b2, b2_d)):
                nc.sync.dma_start(out=dst[:], in_=src[:])
            # big fc1 weight: split across queues
            for i in range(8):
                sl = slice(i * 4096, (i + 1) * 4096)
                nc.sync.dma_start(out=fw[:, sl], in_=fw_d[:, sl])

            # zero-init state (h=0, c=0, halos=0)
            nc.gpsimd.memset(A[:], 0.0)
            nc.gpsimd.memset(Ba[:], 0.0)
            nc.gpsimd.memset(Bb[:], 0.0)
            nc.vector.memset(cst0[:], 0.0)
            nc.vector.memset(cst1[:], 0.0)

            # ---- DRAM bounce buffers for collectives --------------------
            a2a_in = dram.tile([HID, NPIX], F16)
            a2a_out = dram.tile([8, 128, 128], F16)
            z1part = dram.tile([8, 256], F32)
            z1red = dram.tile([256], F32)

            KXS = (-1, 0, 1)

            def pointwise_q(S, cst, hdst, rt):
                """LSTM cell update for one quarter (16 image rows).
                TT inputs must share a base partition, so scratch tensors
                are placed at the base of the gate they pair with."""
                sl = slice(rt * 1024, (rt + 1) * 1024)
                # tg = 2*sigmoid(2g) - 1, re-based to partitions 0-31
                tgt = scr.tile([32, 1024], F16, tag="tgt")
                nc.vector.tensor_scalar(
                    out=tgt[:], in0=S[96:128, sl],
                    scalar1=2.0, scalar2=-1.0, op0=OP.mult, op1=OP.add)
                uv = scr.tile([32, 2, 1024], F16, tag="uv")
                nc.vector.tensor_mul(uv[:, 0, :], S[0:32, sl], tgt[:])
                nc.vector.tensor_mul(uv[:, 1, :], S[32:64, sl],
                                     cst[32:64, sl])
                nc.vector.tensor_add(cst[32:64, sl], uv[:, 0, :],
                                     uv[:, 1, :])
                tht = scr.tile([96, 1024], F16, tag="tht")
                nc.scalar.activation(out=tht[64:96, :],
                                     in_=cst[32:64, sl], func=AF.Tanh)
                if hdst is hdc:
                    dst = hdc[:, sl]
                else:
                    buf, p0 = hdst
                    dst = buf[p0:p0 + 32, 1 + 16 * rt:17 + 16 * rt, 2:66]
                nc.vector.tensor_mul(dst, S[64:96, sl], tht[64:96, :])

            def conv_layer(srcs, bias, S, cst, hdst):
                """srcs: list of (tile, K, weights).  Per row-quarter:
                accumulate all passes into a [128,1024] PSUM tile, sigmoid,
                then that quarter's pointwise — so pointwise overlaps the
                next quarter's matmuls."""
                npass = len(srcs) * 3
                for rt in range(4):
                    pz = psum.tile([G4, 1024], F32, tag="z", name=f"pz{rt}")
                    ip = 0
                    for buf, K, wt in srcs:
                        for kxi, kx in enumerate(KXS):
                            lhs = wt[:, kxi * G4:(kxi + 1) * G4]
                            for h in range(2):
                                r0 = 16 * rt + 8 * h
                                rhs = buf[0:K, r0 + 1:r0 + 9, 2 + kx:66 + kx]
                                nc.tensor.matmul(
                                    pz[:, 512 * h:512 * h + 512],
                                    lhs, rhs, start=(ip == 0),
                                    stop=(ip == npass - 1))
                            ip += 1
                    nc.scalar.activation(out=S[:, rt * 1024:(rt + 1) * 1024],
                                         in_=pz[:], func=AF.Sigmoid,
                                         bias=bias[:, 0:1], scale=1.0)
                    pointwise_q(S, cst, hdst, rt)

            def shift_copies(dsts, src, eng):
                """src: (buf, p0) base-group h [32, PH, PW]; dsts: list of
                (buf, p0, ky).  eng picks the HWDGE queue (sync feeds L0's
                rhs, scalar feeds L1's) to avoid head-of-line blocking."""
                sbuf, sp = src
                for buf, p0, ky in dsts:
                    if ky == 0:
                        eng.dma_start(out=buf[p0:p0 + 32, :, :],
                                      in_=sbuf[sp:sp + 32, :, :])
                    elif ky == -1:
                        eng.dma_start(out=buf[p0:p0 + 32, 1:PH, :],
                                      in_=sbuf[sp:sp + 32, 0:PH - 1, :])
                    else:
                        eng.dma_start(out=buf[p0:p0 + 32, 0:PH - 1, :],
                                      in_=sbuf[sp:sp + 32, 1:PH, :])

            # ================= recurrent steps ===========================
            # Layer 1 runs one step behind layer 0 so the PE alternates
            # between the two layers' matmul bursts with no pointwise gap:
            # L1(t-1)'s inputs (h0(t-1), h1(t-2)) are ready before L0(t)
            # even starts.  The h0(t) -> Ba/Bb copies are emitted AFTER
            # L1(t-1) so Tile's program-order dependency keeps them WAR.
            def l1_step():
                S1 = sgate.tile([G4, NPIX], F16, tag="S", name="S1")
                conv_layer([(Ba, 128, w1a), (Bb, 64, w1b)], b1, S1,
                           cst1, (Ba, 96))
                shift_copies([(Ba, 32, -1), (Bb, 32, 1)], (Ba, 96),
                             nc.scalar)

            def xload(t):
                # x_t into A's 3 ky-groups (ky=0 @32, ky=-1 @67, ky=+1 @102)
                nc.sync.dma_start(out=A[32:35, :, :], in_=xp_d[t])
                nc.sync.dma_start(out=A[67:70, 1:PH, :],
                                  in_=xp_d[t, :, 0:PH - 1, :])
                nc.sync.dma_start(out=A[102:105, 0:PH - 1, :],
                                  in_=xp_d[t, :, 1:PH, :])

            xload(0)
            for t in range(T):
                S0 = sgate.tile([G4, NPIX], F16, tag="S", name="S0")
                conv_layer([(A, 105, w0)], b0, S0, cst0, (A, 0))
                if t + 1 < T:
                    xload(t + 1)       # prefetch; WAR-ordered after L0(t) mms
                # h0(t) shifted copies within A (next L0 step's rhs)
                shift_copies([(A, 35, -1), (A, 70, 1)], (A, 0), nc.sync)

                if t > 0:
                    l1_step()          # L1(t-1)
                # now h0(t) may overwrite L1's rhs state
                shift_copies([(Ba, 64, 0), (Ba, 0, -1), (Bb, 0, 1)], (A, 0),
                             nc.scalar)

            l1_step()                  # L1(T-1)

            # ================= decoder step ==============================
            shift_copies([(A, 0, 0), (A, 35, -1), (A, 70, 1)], (Ba, 96),
                         nc.sync)
            Sd = sgate.tile([G4, NPIX], F16, tag="S")
            conv_layer([(A, 105, wd)], bd, Sd, cst1, hdc)

            # ================= FC head ===================================
            nc.sync.dma_start(out=a2a_in[:], in_=hdc[:])
            nc.gpsimd.collective_compute(
                "AllToAll", OP.bypass,
                replica_groups=[list(range(N_CORES))],
                ins=[a2a_in[:].opt()], outs=[a2a_out[:].opt()])
            # transposed load with K-index q = p*128 + k2:
            # ft[p, m, k2] = a2a_out[m, p, k2] -- contiguous 128-elem runs
            nc.sync.dma_start(
                out=ft[:],
                in_=a2a_out[:].rearrange("m p k -> p m k"))

            psz = psum.tile([8, 256], F32, tag="z")
            for k2 in range(128):
                nc.tensor.matmul(psz[:], ft[:, :, k2],
                                 fw[:, k2 * 256:(k2 + 1) * 256],
                                 start=(k2 == 0), stop=(k2 == 127))
            z1s = scr.tile([8, 256], F32, tag="z1")
            nc.vector.tensor_copy(z1s[:], psz[:])
            nc.sync.dma_start(out=z1part[:], in_=z1s[:])
            nc.gpsimd.collective_compute(
                "ReduceScatter", OP.add,
                replica_groups=[list(range(N_CORES))],
                ins=[z1part[:].opt()], outs=[z1red[:].opt()])

            zr = scr.tile([128, 2], F32, tag="zr")
            nc.sync.dma_start(out=zr[:],
                              in_=z1red[:].rearrange("(j p) -> p j", p=128))
            zrb = scr.tile([128, 2], F32, tag="zrb")
            nc.vector.tensor_add(zrb[:], zr[:], fb[:])
            h256 = scr.tile([128, 2], F16, tag="h256")
            nc.vector.tensor_scalar_max(h256[:], zrb[:], 0.0)

            ps2 = psum.tile([97, 1], F32, tag="z")
            for j in range(2):
                nc.tensor.matmul(ps2[:], w2[:, j * 97:(j + 1) * 97],
                                 h256[:, j:j + 1],
                                 start=(j == 0), stop=(j == 1))
            outs = scr.tile([97, 1], F32, tag="outs")
            nc.vector.tensor_add(outs[:], ps2[:], b2[:])
            nc.sync.dma_start(out=out_d[:], in_=outs[:])

    nc.compile()
    return nc


def _prep_inputs(x, Wenc0, benc0, Wenc1, benc1, Wdec, bdec,
                 fc1_w, fc1_b, fc2_w, fc2_b):
    """Host-side: pad/reorder/cast everything into device layouts."""
    f16 = np.float16

    def conv_w(Wk, reorder_x):
        # Wk [128, Cin, 3, 3] -> per-kx [ngrp*ch, 128] with ky stacked on
        # partitions; gate-g output channels pre-scaled x2.
        Wk = np.asarray(Wk, np.float32).copy()
        Wk[96:128] *= 2.0
        if reorder_x:  # [x(3), h(32)] -> [h(32), x(3)]
            Wk = np.concatenate([Wk[:, 3:], Wk[:, :3]], axis=1)
        cin = Wk.shape[1]
        out = np.zeros((3 * cin, 3 * G4), np.float32)
        for g, dy in enumerate((1, 0, 2)):   # group order ky = 0, -1, +1
            for kxi in range(3):
                # [cin, 128]
                out[g * cin:(g + 1) * cin, kxi * G4:(kxi + 1) * G4] = \
                    Wk[:, :, dy, kxi].T
        return out.astype(f16)

    def bias_v(b):
        b = np.asarray(b, np.float32).copy()
        b[96:128] *= 2.0
        return b.reshape(G4, 1)

    w0_full = conv_w(Wenc0, True)       # [105, 384]
    wd_full = conv_w(Wdec, True)
    w1_full = conv_w(Wenc1, False)      # [192, 384]; groups ky = 0, -1, +1
    # Ba's partition groups are ky=-1 @0-63, ky=0 @64-127
    w1a = np.ascontiguousarray(
        np.concatenate([w1_full[64:128], w1_full[0:64]], axis=0))
    w1b = np.ascontiguousarray(w1_full[128:192])

    xpad = np.zeros((B, T, C, PH, PW), f16)
    xpad[:, :, :, 1:65, 2:66] = np.asarray(x, np.float32)

    fc1_w = np.asarray(fc1_w, np.float32)
    fb = np.asarray(fc1_b, np.float32).reshape(2, 128).T.copy()  # [128, 2]
    w2 = np.asarray(fc2_w, np.float32).T.reshape(2, 128, 97)
    w2 = np.ascontiguousarray(w2.transpose(1, 0, 2)).reshape(128, 2 * 97)
    b2 = np.asarray(fc2_b, np.float32).reshape(97, 1)

    in_maps = []
    for k in range(N_CORES):
        w1k = fc1_w[:, k * KSL:(k + 1) * KSL].T            # [16384, 256]
        # K-index q = p*128 + k2  ->  fw[p, k2, n] = w1k[p*128 + k2, n]
        fwk = w1k.reshape(128, 128 * 256)
        in_maps.append({
            "xp": np.ascontiguousarray(xpad[k]),
            "w0": w0_full, "w1a": w1a.astype(f16), "w1b": w1b.astype(f16),
            "wd": wd_full,
            "b0": bias_v(benc0), "b1": bias_v(benc1), "bd": bias_v(bdec),
            "fw": fwk.astype(f16), "fb": fb,
            "w2": w2.astype(f16), "b2": b2,
        })
    return in_maps


def kernel(**inputs):
    if "nc" not in _CACHE:
        _CACHE["nc"] = _build_nc()
    nc = _CACHE["nc"]
    in_maps = _prep_inputs(**inputs)
    res = run_bass_kernel_spmd(nc, in_maps, core_ids=list(range(N_CORES)),
                               trace=TRACE)
    _CACHE["last_result"] = res
    out = np.stack([res.results[k]["out"][:, 0] for k in range(N_CORES)])
    return out.astype(np.float32)
